# revision 30
# baseline (speedup 1.0000x reference)
"""Trainium2 Bass kernel for a dense transformer decoder layer.

Shapes (hardcoded): B=2, S=2048, D=1024, H=16, HD=64, FF=4096, fp32 I/O.

Strategy: token-parallel over 8 cores (512 tokens each; batch b owned by
cores 4b..4b+3), two SPMD launches with a host-side concat (no arithmetic)
in between:

  Launch 1 (per core, its 512 tokens): LN1 (affine folded into the QKV
  weights on the host) -> Q/K/V projections in bf16 -> outputs qT, kT
  (transposed via the PE) and v shards.

  Launch 2 (per core, its 512 queries + the full K/V of its batch):
  scores^T = K Q^T per head (2-head PE row-tiling), mask folded in as a
  per-partition bias on the ScalarE exp, softmax denominator via an
  appended ones-column on V, ctx -> Wo -> residual -> LN2 -> FFN -> out.

All matmuls run in bf16 with fp32 PSUM accumulation; LN statistics,
residual stream and softmax normalization are fp32.
"""

import os
import numpy as np
import ml_dtypes
from contextlib import ExitStack

import concourse.bass as bass
from concourse import bacc
import concourse.mybir as mybir
import concourse.tile as tile
from concourse.bass_utils import run_bass_kernel_spmd
from concourse.masks import make_identity

B, S, D, H, FF = 2, 2048, 1024, 16, 4096
HD = D // H
EPS = 1e-5
NCORES = 8
TOK = (B * S) // NCORES          # 512 tokens per core
P = 128
DC = D // P                      # 8 contraction chunks
SBLK = TOK // P                  # 4 s-blocks of 128
TCH = S // P                     # 16 t-chunks of 128 per batch
FB = FF // P                     # 32 ff blocks of 128

F32 = mybir.dt.float32
BF16 = mybir.dt.bfloat16
I32 = mybir.dt.int32
AF = mybir.ActivationFunctionType
ALU = mybir.AluOpType

bf16_np = ml_dtypes.bfloat16


def _ln_tile(nc, pools, x_tile, out_tile, eps_sb):
    """LayerNorm (no affine) of one [128, D] fp32 tile into out_tile (bf16)."""
    stats = pools.tile([P, 2, 6], F32, tag="ln_stats")
    mv = pools.tile([P, 2], F32, tag="ln_mv")
    xg = x_tile.rearrange("p (g d) -> p g d", g=2)
    for g in range(2):
        nc.vector.bn_stats(out=stats[:, g, :], in_=xg[:, g, :])
    nc.vector.bn_aggr(out=mv[:], in_=stats[:])
    mean = mv[:, 0:1]
    std = pools.tile([P, 1], F32, tag="ln_std")
    nc.scalar.activation(out=std, in_=mv[:, 1:2], func=AF.Sqrt, bias=eps_sb, scale=1.0)
    nc.vector.reciprocal(out=std, in_=std)
    nc.vector.tensor_scalar(
        out=out_tile,
        in0=x_tile,
        scalar1=mean,
        scalar2=std,
        op0=ALU.subtract,
        op1=ALU.mult,
    )


def _build_launch1():
    nc = bacc.Bacc(None, target_bir_lowering=False, debug=False)
    x_d = nc.declare_dram_parameter("x", [TOK, D], F32, isOutput=False)
    wq_d = nc.declare_dram_parameter("wq", [D, D], BF16, isOutput=False)
    wk_d = nc.declare_dram_parameter("wk", [D, D], BF16, isOutput=False)
    wv_d = nc.declare_dram_parameter("wv", [D, D], BF16, isOutput=False)
    bq_d = nc.declare_dram_parameter("bq", [D], F32, isOutput=False)
    bk_d = nc.declare_dram_parameter("bk", [D], F32, isOutput=False)
    bv_d = nc.declare_dram_parameter("bvb", [P, D], F32, isOutput=False)  # broadcast
    qt_d = nc.declare_dram_parameter("qT", [D, TOK], BF16, isOutput=True)
    kt_d = nc.declare_dram_parameter("kT", [D, TOK], BF16, isOutput=True)
    v_d = nc.declare_dram_parameter("v", [TOK, D], BF16, isOutput=True)

    with tile.TileContext(nc) as tc, ExitStack() as ctx:
        singles = ctx.enter_context(tc.tile_pool(name="singles", bufs=1))
        work = ctx.enter_context(tc.tile_pool(name="work", bufs=3))
        psum = ctx.enter_context(tc.tile_pool(name="psum", bufs=2, space="PSUM"))

        ident = singles.tile([P, P], BF16)
        make_identity(nc, ident)
        eps_sb = singles.tile([P, 1], F32)
        nc.vector.memset(eps_sb, EPS)

        # x tiles first so LN + transposes can start while weights stream in
        x_tiles = []
        for sb in range(SBLK):
            x_tile = work.tile([P, D], F32, tag="x", name=f"x{sb}")
            nc.sync.dma_start(out=x_tile, in_=x_d[sb * P:(sb + 1) * P, :])
            x_tiles.append(x_tile)

        # weights fully resident (2 MB each in bf16), split per-chunk DMAs
        wq_sb = singles.tile([P, DC, D], BF16)
        wk_sb = singles.tile([P, DC, D], BF16)
        wv_sb = singles.tile([P, DC, D], BF16)
        for dc in range(DC):
            nc.gpsimd.dma_start(out=wq_sb[:, dc, :], in_=wq_d[dc * P:(dc + 1) * P, :])
            nc.gpsimd.dma_start(out=wk_sb[:, dc, :], in_=wk_d[dc * P:(dc + 1) * P, :])
            nc.gpsimd.dma_start(out=wv_sb[:, dc, :], in_=wv_d[dc * P:(dc + 1) * P, :])
        bq_sb = singles.tile([P, DC], F32)
        bk_sb = singles.tile([P, DC], F32)
        nc.gpsimd.dma_start(out=bq_sb, in_=bq_d[:].rearrange("(c p) -> p c", p=P))
        nc.gpsimd.dma_start(out=bk_sb, in_=bk_d[:].rearrange("(c p) -> p c", p=P))
        bv_sb = singles.tile([P, D], F32)
        nc.gpsimd.dma_start(out=bv_sb, in_=bv_d[:])

        # LN1 over the 4 s-blocks, then transpose into z1T [128, DC, TOK]
        z1t_sb = singles.tile([P, DC, TOK], BF16)
        for sb in range(SBLK):
            z1 = work.tile([P, D], BF16, tag="z1")
            _ln_tile(nc, work, x_tiles[sb], z1, eps_sb)
            for dc in range(DC):
                pt = psum.tile([P, P], BF16, tag="tp")
                nc.tensor.transpose(pt, z1[:, dc * P:(dc + 1) * P], ident)
                nc.vector.tensor_copy(
                    out=z1t_sb[:, dc, sb * P:(sb + 1) * P], in_=pt)

        # qT / kT: out block [hd_block 128, TOK] = sum_dc W[dc][:, hb].T @ z1T[dc]
        for (w_sb, b_sb, out_d) in ((wq_sb, bq_sb, qt_d), (wk_sb, bk_sb, kt_d)):
            for hb in range(DC):
                pq = psum.tile([P, TOK], F32, tag="pqk")
                for dc in range(DC):
                    nc.tensor.matmul(
                        pq,
                        lhsT=w_sb[:, dc, hb * P:(hb + 1) * P],
                        rhs=z1t_sb[:, dc, :],
                        start=(dc == 0),
                        stop=(dc == DC - 1),
                    )
                ot = work.tile([P, TOK], BF16, tag="qk_out")
                nc.vector.tensor_scalar_add(out=ot, in0=pq, scalar1=b_sb[:, hb:hb + 1])
                nc.sync.dma_start(out=out_d[hb * P:(hb + 1) * P, :], in_=ot)

        # v natural: [s_block 128, D] = sum_dc z1T[dc][:, sblk].T @ Wv[dc]
        for sb in range(SBLK):
            pv = psum.tile([P, D], F32, tag="pv")
            for dc in range(DC):
                for vh in range(2):
                    nc.tensor.matmul(
                        pv[:, vh * 512:(vh + 1) * 512],
                        lhsT=z1t_sb[:, dc, sb * P:(sb + 1) * P],
                        rhs=wv_sb[:, dc, vh * 512:(vh + 1) * 512],
                        start=(dc == 0),
                        stop=(dc == DC - 1),
                    )
            vt = work.tile([P, D], BF16, tag="v_out")
            nc.vector.tensor_add(out=vt, in0=pv, in1=bv_sb)
            nc.sync.dma_start(out=v_d[sb * P:(sb + 1) * P, :], in_=vt)

    return nc


def _build_launch2(TKEY=S):
    nc = bacc.Bacc(None, target_bir_lowering=False, debug=False)
    qt_d = nc.declare_dram_parameter("qT", [D, TOK], BF16, isOutput=False)
    kt_d = nc.declare_dram_parameter("kT", [D, TKEY], BF16, isOutput=False)
    v_d = nc.declare_dram_parameter("v", [TKEY, D], BF16, isOutput=False)
    mask_d = nc.declare_dram_parameter("mask", [TKEY], I32, isOutput=False)
    x_d = nc.declare_dram_parameter("x", [TOK, D], F32, isOutput=False)
    wo_d = nc.declare_dram_parameter("wo", [D, D], BF16, isOutput=False)
    bo_d = nc.declare_dram_parameter("bob", [P, D], F32, isOutput=False)   # broadcast
    w1_d = nc.declare_dram_parameter("w1", [D, FF], BF16, isOutput=False)
    b1_d = nc.declare_dram_parameter("b1", [FF], F32, isOutput=False)
    w2_d = nc.declare_dram_parameter("w2", [FF, D], BF16, isOutput=False)
    b2_d = nc.declare_dram_parameter("b2b", [P, D], F32, isOutput=False)   # broadcast
    out_d = nc.declare_dram_parameter("out", [TOK, D], F32, isOutput=True)

    TCHL = TKEY // P
    with tile.TileContext(nc) as tc, ExitStack() as ctx:
        glob = ctx.enter_context(tc.tile_pool(name="glob", bufs=1))

        ident = glob.tile([P, P], BF16)
        make_identity(nc, ident)
        eps_sb = glob.tile([P, 1], F32)
        nc.vector.memset(eps_sb, EPS)
        ones_sb = glob.tile([1, 64], BF16)
        nc.vector.memset(ones_sb, 1.0)

        # --- global loads ---
        bo_sb = glob.tile([P, D], F32)
        nc.gpsimd.dma_start(out=bo_sb, in_=bo_d[:])
        b2_sb = glob.tile([P, D], F32)
        nc.gpsimd.dma_start(out=b2_sb, in_=b2_d[:])
        b1_sb = glob.tile([P, FB], F32)
        nc.gpsimd.dma_start(out=b1_sb, in_=b1_d[:].rearrange("(c p) -> p c", p=P))

        # mask -> additive bias (-1e9 where mask==0), laid out [p, t_chunk]
        mask_i = glob.tile([P, TCHL], I32)
        nc.sync.dma_start(out=mask_i, in_=mask_d[:].rearrange("(t p) -> p t", p=P))
        maskb_sb = glob.tile([P, TCHL], F32)
        nc.vector.tensor_copy(out=maskb_sb, in_=mask_i)  # int -> float
        nc.vector.tensor_scalar(
            out=maskb_sb, in0=maskb_sb, scalar1=1.0, scalar2=1e9,
            op0=ALU.subtract, op1=ALU.mult)

        # --- attention ---
        ctx_sb = glob.tile([P, DC, TOK], BF16)       # ctx^T (normalized in place)
        x_sb = glob.tile([P, SBLK, D], F32)
        wo_sb = glob.tile([P, DC, D], BF16)

        with tc.tile_pool(name="attn_in", bufs=1) as ain, \
             tc.tile_pool(name="attn_exp", bufs=4) as exp_pool, \
             tc.tile_pool(name="attn_sc", bufs=3, space="PSUM") as sc_psum, \
             tc.tile_pool(name="attn_ctx", bufs=2, space="PSUM") as ctx_psum, \
             tc.tile_pool(name="attn_wk", bufs=2) as awork:
            qt_sb = ain.tile([P, DC, TOK], BF16)
            kt_sb = ain.tile([P, DC, TKEY], BF16)
            ctxu_sb = ain.tile([P, DC, TOK], BF16)   # unnormalized ctx^T
            vaug_sb = ain.tile([P, TCHL, H, HD + 1], BF16)
            # pair 0's inputs first so its scores start immediately
            nc.sync.dma_start(out=qt_sb[:, 0, :], in_=qt_d[0:P, :])
            nc.sync.dma_start(out=kt_sb[:, 0, :], in_=kt_d[0:P, :])
            nc.vector.memset(vaug_sb[:, :, :, HD:HD + 1], 1.0)  # ones-column only
            for tc_i in range(TCHL):
                vdense = awork.tile([P, D], BF16, tag="vdense")
                nc.gpsimd.dma_start(out=vdense,
                                    in_=v_d[tc_i * P:(tc_i + 1) * P, :])
                nc.vector.tensor_copy(
                    out=vaug_sb[:, tc_i, :, 0:HD],
                    in_=vdense.rearrange("p (h k) -> p h k", h=H))
            for pair in range(1, H // 2):
                nc.sync.dma_start(out=qt_sb[:, pair, :],
                                  in_=qt_d[pair * P:(pair + 1) * P, :])
                nc.sync.dma_start(out=kt_sb[:, pair, :],
                                  in_=kt_d[pair * P:(pair + 1) * P, :])
            nc.gpsimd.dma_start(out=x_sb, in_=x_d[:].rearrange("(s p) d -> p s d", p=P))
            nc.gpsimd.dma_start(out=wo_sb, in_=wo_d[:].rearrange("(c p) n -> p c n", p=P))
            for pair in range(H // 2):
                # scores^T for both heads into one 2-bank psum tile per chunk,
                # exp'd by a single ACT op (mask bias is per-partition = per-t)
                et_halves = [exp_pool.tile([P, (TCHL + 1) // 2, 2 * TOK], BF16,
                                           tag="exp", name=f"et{pair}_{i}")
                             for i in range(2)]
                half = (TCHL + 1) // 2
                for tb in range(TCHL):
                    et = et_halves[tb // half]
                    ti = tb % half
                    ps = sc_psum.tile([P, 2 * TOK], F32, tag="sc")
                    for hi in range(2):
                        po = 64 * hi
                        nc.tensor.matmul(
                            ps[:, hi * TOK:(hi + 1) * TOK],
                            lhsT=kt_sb[po:po + 64, pair, tb * P:(tb + 1) * P],
                            rhs=qt_sb[po:po + 64, pair, :],
                            start=True, stop=True,
                            tile_position=(po, 0),
                        )
                    nc.scalar.activation(
                        out=et[:, ti, :], in_=ps, func=AF.Exp,
                        bias=maskb_sb[:, tb:tb + 1], scale=1.0)
                denstage = awork.tile([1, 2, TOK], F32, tag="denstage")
                for hi in range(2):
                    h = pair * 2 + hi
                    pc = ctx_psum.tile([HD + 1, TOK], F32, tag="ctx")
                    for tb in range(TCHL):
                        et = et_halves[tb // half]
                        ti = tb % half
                        nc.tensor.matmul(
                            pc,
                            lhsT=vaug_sb[:, tb, h, :],
                            rhs=et[:, ti, hi * TOK:(hi + 1) * TOK],
                            start=(tb == 0), stop=(tb == TCHL - 1),
                        )
                    po = 64 * hi
                    nc.vector.tensor_copy(
                        out=ctxu_sb[po:po + 64, pair, :], in_=pc[0:HD, :])
                    nc.vector.tensor_copy(out=denstage[:, hi, :],
                                          in_=pc[HD:HD + 1, :])
                # per-pair softmax normalization, pipelined behind the next
                # pair's scores: reciprocal runs on 2 partitions, broadcast
                # across 64 partitions via a K=1 outer product on the PE
                den2 = awork.tile([2, TOK], F32, tag="den2")
                nc.sync.dma_start(out=den2, in_=denstage)
                nc.vector.reciprocal(out=den2, in_=den2)
                rcb = awork.tile([2, TOK], BF16, tag="rcb")
                nc.vector.tensor_copy(out=rcb, in_=den2)
                rcflat = awork.tile([1, 2, TOK], BF16, tag="rcflat")
                nc.sync.dma_start(out=rcflat, in_=rcb)
                pb = ctx_psum.tile([P, TOK], F32, tag="ctx")
                for hi in range(2):
                    po = 64 * hi
                    nc.tensor.matmul(pb[po:po + 64, :], lhsT=ones_sb,
                                     rhs=rcflat[:, hi, :],
                                     start=True, stop=True,
                                     tile_position=(0, po))
                rb_sb = awork.tile([P, TOK], F32, tag="rbsb")
                nc.vector.tensor_copy(out=rb_sb, in_=pb)
                nc.vector.tensor_mul(
                    out=ctx_sb[:, pair, :],
                    in0=ctxu_sb[:, pair, :], in1=rb_sb)

        # --- Wo + residual + LN2 ---
        mid = ctx.enter_context(tc.tile_pool(name="mid", bufs=1))
        resid_sb = mid.tile([P, SBLK, D], F32)
        z2t_sb = mid.tile([P, DC, TOK], BF16)
        with tc.tile_pool(name="wo_psum", bufs=2, space="PSUM") as wo_psum, \
             tc.tile_pool(name="wo_wk", bufs=3) as wwork:
            for sb in range(SBLK):
                pw = wo_psum.tile([P, D], F32, tag="wo")
                for dc in range(DC):
                    for oh in range(2):
                        nc.tensor.matmul(
                            pw[:, oh * 512:(oh + 1) * 512],
                            lhsT=ctx_sb[:, dc, sb * P:(sb + 1) * P],
                            rhs=wo_sb[:, dc, oh * 512:(oh + 1) * 512],
                            start=(dc == 0), stop=(dc == DC - 1),
                        )
                rs = resid_sb[:, sb, :]
                nc.vector.tensor_add(out=rs, in0=pw, in1=x_sb[:, sb, :])
                nc.vector.tensor_add(out=rs, in0=rs, in1=bo_sb)
                z2 = wwork.tile([P, D], BF16, tag="z2")
                _ln_tile(nc, wwork, rs, z2, eps_sb)
                for dc in range(DC):
                    pt = wo_psum.tile([P, P], BF16, tag="tp2")
                    nc.tensor.transpose(pt, z2[:, dc * P:(dc + 1) * P], ident)
                    nc.vector.tensor_copy(
                        out=z2t_sb[:, dc, sb * P:(sb + 1) * P], in_=pt)

        # --- FFN (fused: h1 block feeds the second GEMM immediately) ---
        ffn = ctx.enter_context(tc.tile_pool(name="ffn", bufs=1))
        h1t_sb = ffn.tile([P, FB, TOK], BF16)
        with tc.tile_pool(name="ff_psum", bufs=2, space="PSUM") as fa_psum, \
             tc.tile_pool(name="ffb_psum", bufs=4, space="PSUM") as fb_psum, \
             tc.tile_pool(name="ff_w1", bufs=6) as w1pool, \
             tc.tile_pool(name="ff_w2", bufs=6) as w2pool, \
             tc.tile_pool(name="ff_wk", bufs=4) as fwork:
            po_tiles = [fb_psum.tile([P, 512], F32, tag="ffb", name=f"po0_{sb}")
                        for sb in range(SBLK)]
            for fb in range(FB):
                w1t = w1pool.tile([P, DC, P], BF16, tag="w1t")
                nc.gpsimd.dma_start(
                    out=w1t,
                    in_=w1_d[:, fb * P:(fb + 1) * P].rearrange(
                        "(c p) f -> p c f", p=P))
                pf = fa_psum.tile([P, TOK], F32, tag="ffa")
                for dc in range(DC):
                    nc.tensor.matmul(
                        pf, lhsT=w1t[:, dc, :], rhs=z2t_sb[:, dc, :],
                        start=(dc == 0), stop=(dc == DC - 1))
                nc.scalar.activation(
                    out=h1t_sb[:, fb, :], in_=pf, func=AF.Relu,
                    bias=b1_sb[:, fb:fb + 1], scale=1.0)
                w2t = w2pool.tile([P, 512], BF16, tag="w2t")
                nc.gpsimd.dma_start(out=w2t, in_=w2_d[fb * P:(fb + 1) * P, 0:512])
                for sb in range(SBLK):
                    nc.tensor.matmul(
                        po_tiles[sb], lhsT=h1t_sb[:, fb, sb * P:(sb + 1) * P],
                        rhs=w2t, start=(fb == 0), stop=(fb == FB - 1))
            for sb in range(SBLK):
                ot = fwork.tile([P, 512], F32, tag="out")
                nc.vector.tensor_add(out=ot, in0=po_tiles[sb],
                                     in1=resid_sb[:, sb, 0:512])
                nc.vector.tensor_add(out=ot, in0=ot, in1=b2_sb[:, 0:512])
                nc.sync.dma_start(out=out_d[sb * P:(sb + 1) * P, 0:512], in_=ot)
            # second output half: re-stream W2's right half
            po2_tiles = [fb_psum.tile([P, 512], F32, tag="ffb", name=f"po1_{sb}")
                         for sb in range(SBLK)]
            for fb in range(FB):
                w2t = w2pool.tile([P, 512], BF16, tag="w2t")
                nc.gpsimd.dma_start(out=w2t, in_=w2_d[fb * P:(fb + 1) * P, 512:1024])
                for sb in range(SBLK):
                    nc.tensor.matmul(
                        po2_tiles[sb], lhsT=h1t_sb[:, fb, sb * P:(sb + 1) * P],
                        rhs=w2t, start=(fb == 0), stop=(fb == FB - 1))
            for sb in range(SBLK):
                ot = fwork.tile([P, 512], F32, tag="out")
                nc.vector.tensor_add(out=ot, in0=po2_tiles[sb],
                                     in1=resid_sb[:, sb, 512:1024])
                nc.vector.tensor_add(out=ot, in0=ot, in1=b2_sb[:, 512:1024])
                nc.sync.dma_start(out=out_d[sb * P:(sb + 1) * P, 512:1024], in_=ot)

    return nc


_programs = {}
LAST_EXEC_NS = {}


T_PAD = 1280  # compacted key columns (only mask==1 keys kept, padded)


def _get_programs(tkey):
    if "l1" not in _programs:
        l1 = _build_launch1()
        l1.finalize()
        _programs["l1"] = l1
    if ("l2", tkey) not in _programs:
        l2 = _build_launch2(tkey)
        l2.finalize()
        _programs[("l2", tkey)] = l2
    return _programs["l1"], _programs[("l2", tkey)]


def kernel(**inputs):
    inp = {k: np.asarray(v) for k, v in inputs.items()}
    x = inp["x"].astype(np.float32).reshape(B * S, D)
    mask = inp["mask"].astype(np.int32)

    # ---- host-side weight prep (layout + LN-affine folding, fp32 math) ----
    scale = np.float32(1.0 / np.sqrt(HD))
    Wq = inp["Wq"].astype(np.float32).transpose(1, 0, 2).reshape(D, D)
    Wk = inp["Wk"].astype(np.float32).transpose(1, 0, 2).reshape(D, D)
    Wv = inp["Wv"].astype(np.float32).transpose(1, 0, 2).reshape(D, D)
    g1 = inp["ln1_g"].astype(np.float32)
    b1n = inp["ln1_b"].astype(np.float32)
    g2 = inp["ln2_g"].astype(np.float32)
    b2n = inp["ln2_b"].astype(np.float32)

    wq_p = np.ascontiguousarray((g1[:, None] * Wq * scale).astype(bf16_np))
    bq_p = np.ascontiguousarray(
        (b1n @ Wq) * scale + inp["bq"].astype(np.float32).reshape(-1) * scale
    ).astype(np.float32)
    wk_p = np.ascontiguousarray((g1[:, None] * Wk).astype(bf16_np))
    bk_p = ((b1n @ Wk) + inp["bk"].astype(np.float32).reshape(-1)).astype(np.float32)
    wv_p = np.ascontiguousarray((g1[:, None] * Wv).astype(bf16_np))
    bv_p = ((b1n @ Wv) + inp["bv"].astype(np.float32).reshape(-1)).astype(np.float32)
    bv_b = np.ascontiguousarray(np.tile(bv_p[None, :], (P, 1)))

    wo_p = np.ascontiguousarray(inp["Wo"].astype(np.float32).astype(bf16_np))
    bo_b = np.ascontiguousarray(
        np.tile(inp["bo"].astype(np.float32)[None, :], (P, 1)))
    w1_p = np.ascontiguousarray(
        (g2[:, None] * inp["W1"].astype(np.float32)).astype(bf16_np))
    b1_p = ((b2n @ inp["W1"].astype(np.float32))
            + inp["b1"].astype(np.float32)).astype(np.float32)
    w2_p = np.ascontiguousarray(inp["W2"].astype(np.float32).astype(bf16_np))
    b2_b = np.ascontiguousarray(
        np.tile(inp["b2"].astype(np.float32)[None, :], (P, 1)))

    # compaction: keys with mask==0 contribute exactly zero (exp(-1e9) == 0),
    # so only mask==1 columns are kept, padded to T_PAD with mask==0 dummies.
    counts = [int((mask[b] == 1).sum()) for b in range(B)]
    tkey = T_PAD if max(counts) <= T_PAD else S
    l1, l2 = _get_programs(tkey)
    core_ids = list(range(NCORES))
    profile = bool(os.environ.get("KERNEL_PROFILE"))
    kw = {"trace": True} if profile else {}

    # ---- launch 1 ----
    in_maps1 = []
    for c in range(NCORES):
        xc = np.ascontiguousarray(x[c * TOK:(c + 1) * TOK, :])
        in_maps1.append({
            "x": xc, "wq": wq_p, "wk": wk_p, "wv": wv_p,
            "bq": bq_p, "bk": bk_p, "bvb": bv_b,
        })
    r1 = run_bass_kernel_spmd(l1, in_maps1, core_ids, **kw)
    res1 = r1.results

    # ---- host reshuffle: concat K/V shards per batch, compact by mask ----
    kT_b, v_b, mask_b = [], [], []
    for b in range(B):
        kT = np.concatenate([res1[4 * b + i]["kT"] for i in range(4)], axis=1)
        v = np.concatenate([res1[4 * b + i]["v"] for i in range(4)], axis=0)
        if tkey == S:
            kT_b.append(np.ascontiguousarray(kT))
            v_b.append(np.ascontiguousarray(v))
            mask_b.append(np.ascontiguousarray(mask[b]))
        else:
            idx = np.nonzero(mask[b] == 1)[0]
            pad = tkey - len(idx)
            idx_pad = np.concatenate([idx, np.zeros(pad, np.int64)])
            kT_b.append(np.ascontiguousarray(kT[:, idx_pad]))
            v_b.append(np.ascontiguousarray(v[idx_pad, :]))
            mask_b.append(np.concatenate(
                [np.ones(len(idx), np.int32), np.zeros(pad, np.int32)]))

    # ---- launch 2 ----
    in_maps2 = []
    for c in range(NCORES):
        b = c // 4
        in_maps2.append({
            "qT": np.ascontiguousarray(res1[c]["qT"]),
            "kT": kT_b[b],
            "v": v_b[b],
            "mask": mask_b[b],
            "x": np.ascontiguousarray(x[c * TOK:(c + 1) * TOK, :]),
            "wo": wo_p, "bob": bo_b,
            "w1": w1_p, "b1": b1_p, "w2": w2_p, "b2b": b2_b,
        })
    r2 = run_bass_kernel_spmd(l2, in_maps2, core_ids, **kw)
    res2 = r2.results

    if profile:
        LAST_EXEC_NS.clear()
        LAST_EXEC_NS["l1"] = r1.exec_time_ns
        LAST_EXEC_NS["l2"] = r2.exec_time_ns
        LAST_EXEC_NS["l1_trace"] = getattr(r1, "instructions_and_trace", None)
        LAST_EXEC_NS["l2_trace"] = getattr(r2, "instructions_and_trace", None)

    out = np.concatenate([res2[c]["out"] for c in range(NCORES)], axis=0)
    return out.reshape(B, S, D).astype(np.float32)


# revision 31
# speedup vs baseline: 1.0093x; 1.0093x over previous
"""Trainium2 Bass kernel for a dense transformer decoder layer.

Shapes (hardcoded): B=2, S=2048, D=1024, H=16, HD=64, FF=4096, fp32 I/O.

Strategy: token-parallel over 8 cores (512 tokens each; batch b owned by
cores 4b..4b+3), two SPMD launches with a host-side concat (no arithmetic)
in between:

  Launch 1 (per core, its 512 tokens): LN1 (affine folded into the QKV
  weights on the host) -> Q/K/V projections in bf16 -> outputs qT, kT
  (transposed via the PE) and v shards.

  Launch 2 (per core, its 512 queries + the full K/V of its batch):
  scores^T = K Q^T per head (2-head PE row-tiling), mask folded in as a
  per-partition bias on the ScalarE exp, softmax denominator via an
  appended ones-column on V, ctx -> Wo -> residual -> LN2 -> FFN -> out.

All matmuls run in bf16 with fp32 PSUM accumulation; LN statistics,
residual stream and softmax normalization are fp32.
"""

import os
import numpy as np
import ml_dtypes
from contextlib import ExitStack

import concourse.bass as bass
from concourse import bacc
import concourse.mybir as mybir
import concourse.tile as tile
from concourse.bass_utils import run_bass_kernel_spmd
from concourse.masks import make_identity

B, S, D, H, FF = 2, 2048, 1024, 16, 4096
HD = D // H
EPS = 1e-5
NCORES = 8
TOK = (B * S) // NCORES          # 512 tokens per core
P = 128
DC = D // P                      # 8 contraction chunks
SBLK = TOK // P                  # 4 s-blocks of 128
TCH = S // P                     # 16 t-chunks of 128 per batch
FB = FF // P                     # 32 ff blocks of 128

F32 = mybir.dt.float32
BF16 = mybir.dt.bfloat16
I32 = mybir.dt.int32
AF = mybir.ActivationFunctionType
ALU = mybir.AluOpType

bf16_np = ml_dtypes.bfloat16


def _ln_tile(nc, pools, x_tile, out_tile, eps_sb):
    """LayerNorm (no affine) of one [128, D] fp32 tile into out_tile (bf16)."""
    stats = pools.tile([P, 2, 6], F32, tag="ln_stats")
    mv = pools.tile([P, 2], F32, tag="ln_mv")
    xg = x_tile.rearrange("p (g d) -> p g d", g=2)
    for g in range(2):
        nc.vector.bn_stats(out=stats[:, g, :], in_=xg[:, g, :])
    nc.vector.bn_aggr(out=mv[:], in_=stats[:])
    mean = mv[:, 0:1]
    std = pools.tile([P, 1], F32, tag="ln_std")
    nc.scalar.activation(out=std, in_=mv[:, 1:2], func=AF.Sqrt, bias=eps_sb, scale=1.0)
    nc.vector.reciprocal(out=std, in_=std)
    nc.vector.tensor_scalar(
        out=out_tile,
        in0=x_tile,
        scalar1=mean,
        scalar2=std,
        op0=ALU.subtract,
        op1=ALU.mult,
    )


def _build_launch1():
    nc = bacc.Bacc(None, target_bir_lowering=False, debug=False)
    x_d = nc.declare_dram_parameter("x", [TOK, D], F32, isOutput=False)
    wq_d = nc.declare_dram_parameter("wq", [D, D], BF16, isOutput=False)
    wk_d = nc.declare_dram_parameter("wk", [D, D], BF16, isOutput=False)
    wv_d = nc.declare_dram_parameter("wv", [D, D], BF16, isOutput=False)
    bq_d = nc.declare_dram_parameter("bq", [D], F32, isOutput=False)
    bk_d = nc.declare_dram_parameter("bk", [D], F32, isOutput=False)
    bv_d = nc.declare_dram_parameter("bvb", [P, D], F32, isOutput=False)  # broadcast
    qt_d = nc.declare_dram_parameter("qT", [D, TOK], BF16, isOutput=True)
    kt_d = nc.declare_dram_parameter("kT", [D, TOK], BF16, isOutput=True)
    v_d = nc.declare_dram_parameter("v", [TOK, D], BF16, isOutput=True)

    with tile.TileContext(nc) as tc, ExitStack() as ctx:
        singles = ctx.enter_context(tc.tile_pool(name="singles", bufs=1))
        work = ctx.enter_context(tc.tile_pool(name="work", bufs=3))
        psum = ctx.enter_context(tc.tile_pool(name="psum", bufs=2, space="PSUM"))

        ident = singles.tile([P, P], BF16)
        make_identity(nc, ident)
        eps_sb = singles.tile([P, 1], F32)
        nc.vector.memset(eps_sb, EPS)

        # x tiles first so LN + transposes can start while weights stream in
        x_tiles = []
        for sb in range(SBLK):
            x_tile = work.tile([P, D], F32, tag="x", name=f"x{sb}")
            nc.sync.dma_start(out=x_tile, in_=x_d[sb * P:(sb + 1) * P, :])
            x_tiles.append(x_tile)

        # weights fully resident (2 MB each in bf16), split per-chunk DMAs
        wq_sb = singles.tile([P, DC, D], BF16)
        wk_sb = singles.tile([P, DC, D], BF16)
        wv_sb = singles.tile([P, DC, D], BF16)
        for dc in range(DC):
            nc.gpsimd.dma_start(out=wq_sb[:, dc, :], in_=wq_d[dc * P:(dc + 1) * P, :])
            nc.gpsimd.dma_start(out=wk_sb[:, dc, :], in_=wk_d[dc * P:(dc + 1) * P, :])
            nc.gpsimd.dma_start(out=wv_sb[:, dc, :], in_=wv_d[dc * P:(dc + 1) * P, :])
        bq_sb = singles.tile([P, DC], F32)
        bk_sb = singles.tile([P, DC], F32)
        nc.gpsimd.dma_start(out=bq_sb, in_=bq_d[:].rearrange("(c p) -> p c", p=P))
        nc.gpsimd.dma_start(out=bk_sb, in_=bk_d[:].rearrange("(c p) -> p c", p=P))
        bv_sb = singles.tile([P, D], F32)
        nc.gpsimd.dma_start(out=bv_sb, in_=bv_d[:])

        # LN1 over the 4 s-blocks, then transpose into z1T [128, DC, TOK]
        z1t_sb = singles.tile([P, DC, TOK], BF16)
        for sb in range(SBLK):
            z1 = work.tile([P, D], BF16, tag="z1")
            _ln_tile(nc, work, x_tiles[sb], z1, eps_sb)
            for dc in range(DC):
                pt = psum.tile([P, P], BF16, tag="tp")
                nc.tensor.transpose(pt, z1[:, dc * P:(dc + 1) * P], ident)
                nc.vector.tensor_copy(
                    out=z1t_sb[:, dc, sb * P:(sb + 1) * P], in_=pt)

        # qT / kT: out block [hd_block 128, TOK] = sum_dc W[dc][:, hb].T @ z1T[dc]
        for (w_sb, b_sb, out_d) in ((wq_sb, bq_sb, qt_d), (wk_sb, bk_sb, kt_d)):
            for hb in range(DC):
                pq = psum.tile([P, TOK], F32, tag="pqk")
                for dc in range(DC):
                    nc.tensor.matmul(
                        pq,
                        lhsT=w_sb[:, dc, hb * P:(hb + 1) * P],
                        rhs=z1t_sb[:, dc, :],
                        start=(dc == 0),
                        stop=(dc == DC - 1),
                    )
                ot = work.tile([P, TOK], BF16, tag="qk_out")
                nc.vector.tensor_scalar_add(out=ot, in0=pq, scalar1=b_sb[:, hb:hb + 1])
                nc.sync.dma_start(out=out_d[hb * P:(hb + 1) * P, :], in_=ot)

        # v natural: [s_block 128, D] = sum_dc z1T[dc][:, sblk].T @ Wv[dc]
        for sb in range(SBLK):
            pv = psum.tile([P, D], F32, tag="pv")
            for dc in range(DC):
                for vh in range(2):
                    nc.tensor.matmul(
                        pv[:, vh * 512:(vh + 1) * 512],
                        lhsT=z1t_sb[:, dc, sb * P:(sb + 1) * P],
                        rhs=wv_sb[:, dc, vh * 512:(vh + 1) * 512],
                        start=(dc == 0),
                        stop=(dc == DC - 1),
                    )
            vt = work.tile([P, D], BF16, tag="v_out")
            nc.vector.tensor_add(out=vt, in0=pv, in1=bv_sb)
            nc.sync.dma_start(out=v_d[sb * P:(sb + 1) * P, :], in_=vt)

    return nc


def _build_launch2(TKEY=S):
    nc = bacc.Bacc(None, target_bir_lowering=False, debug=False)
    qt_d = nc.declare_dram_parameter("qT", [D, TOK], BF16, isOutput=False)
    kt_d = nc.declare_dram_parameter("kT", [D, TKEY], BF16, isOutput=False)
    v_d = nc.declare_dram_parameter("v", [TKEY, D], BF16, isOutput=False)
    mask_d = nc.declare_dram_parameter("mask", [TKEY], I32, isOutput=False)
    x_d = nc.declare_dram_parameter("x", [TOK, D], F32, isOutput=False)
    wo_d = nc.declare_dram_parameter("wo", [D, D], BF16, isOutput=False)
    bo_d = nc.declare_dram_parameter("bob", [P, D], F32, isOutput=False)   # broadcast
    w1_d = nc.declare_dram_parameter("w1", [D, FF], BF16, isOutput=False)
    b1_d = nc.declare_dram_parameter("b1", [FF], F32, isOutput=False)
    w2_d = nc.declare_dram_parameter("w2", [FF, D], BF16, isOutput=False)
    b2_d = nc.declare_dram_parameter("b2b", [P, D], F32, isOutput=False)   # broadcast
    out_d = nc.declare_dram_parameter("out", [TOK, D], F32, isOutput=True)

    TCHL = TKEY // P
    with tile.TileContext(nc) as tc, ExitStack() as ctx:
        glob = ctx.enter_context(tc.tile_pool(name="glob", bufs=1))

        ident = glob.tile([P, P], BF16)
        make_identity(nc, ident)
        eps_sb = glob.tile([P, 1], F32)
        nc.vector.memset(eps_sb, EPS)
        ones_sb = glob.tile([1, 64], BF16)
        nc.vector.memset(ones_sb, 1.0)

        # --- global loads ---
        bo_sb = glob.tile([P, D], F32)
        nc.gpsimd.dma_start(out=bo_sb, in_=bo_d[:])
        b2_sb = glob.tile([P, D], F32)
        nc.gpsimd.dma_start(out=b2_sb, in_=b2_d[:])
        b1_sb = glob.tile([P, FB], F32)
        nc.gpsimd.dma_start(out=b1_sb, in_=b1_d[:].rearrange("(c p) -> p c", p=P))

        # mask -> additive bias (-1e9 where mask==0), laid out [p, t_chunk]
        mask_i = glob.tile([P, TCHL], I32)
        nc.sync.dma_start(out=mask_i, in_=mask_d[:].rearrange("(t p) -> p t", p=P))
        maskb_sb = glob.tile([P, TCHL], F32)
        nc.vector.tensor_copy(out=maskb_sb, in_=mask_i)  # int -> float
        nc.vector.tensor_scalar(
            out=maskb_sb, in0=maskb_sb, scalar1=1.0, scalar2=1e9,
            op0=ALU.subtract, op1=ALU.mult)

        # --- attention ---
        ctx_sb = glob.tile([P, DC, TOK], BF16)       # ctx^T (normalized in place)
        x_sb = glob.tile([P, SBLK, D], F32)
        wo_sb = glob.tile([P, DC, D], BF16)

        with tc.tile_pool(name="attn_in", bufs=1) as ain, \
             tc.tile_pool(name="attn_exp", bufs=4) as exp_pool, \
             tc.tile_pool(name="attn_sc", bufs=3, space="PSUM") as sc_psum, \
             tc.tile_pool(name="attn_ctx", bufs=2, space="PSUM") as ctx_psum, \
             tc.tile_pool(name="attn_wk", bufs=2) as awork:
            qt_sb = ain.tile([P, DC, TOK], BF16)
            kt_sb = ain.tile([P, DC, TKEY], BF16)
            ctxu_sb = ain.tile([P, DC, TOK], BF16)   # unnormalized ctx^T
            vaug_sb = ain.tile([P, TCHL, H, HD + 1], BF16)
            # pair 0's inputs first so its scores start immediately
            nc.sync.dma_start(out=qt_sb[:, 0, :], in_=qt_d[0:P, :])
            nc.sync.dma_start(out=kt_sb[:, 0, :], in_=kt_d[0:P, :])
            nc.vector.memset(vaug_sb[:, :, :, HD:HD + 1], 1.0)  # ones-column only
            for tc_i in range(TCHL):
                vdense = awork.tile([P, D], BF16, tag="vdense")
                nc.gpsimd.dma_start(out=vdense,
                                    in_=v_d[tc_i * P:(tc_i + 1) * P, :])
                nc.vector.tensor_copy(
                    out=vaug_sb[:, tc_i, :, 0:HD],
                    in_=vdense.rearrange("p (h k) -> p h k", h=H))
            for pair in range(1, H // 2):
                nc.sync.dma_start(out=qt_sb[:, pair, :],
                                  in_=qt_d[pair * P:(pair + 1) * P, :])
                nc.sync.dma_start(out=kt_sb[:, pair, :],
                                  in_=kt_d[pair * P:(pair + 1) * P, :])
            nc.gpsimd.dma_start(out=x_sb, in_=x_d[:].rearrange("(s p) d -> p s d", p=P))
            nc.gpsimd.dma_start(out=wo_sb, in_=wo_d[:].rearrange("(c p) n -> p c n", p=P))
            for pair in range(H // 2):
                # scores^T for both heads into one 2-bank psum tile per chunk,
                # exp'd by a single ACT op (mask bias is per-partition = per-t)
                et_halves = [exp_pool.tile([P, (TCHL + 1) // 2, 2 * TOK], BF16,
                                           tag="exp", name=f"et{pair}_{i}")
                             for i in range(2)]
                half = (TCHL + 1) // 2
                for tb in range(TCHL):
                    et = et_halves[tb // half]
                    ti = tb % half
                    ps = sc_psum.tile([P, 2 * TOK], F32, tag="sc")
                    for hi in range(2):
                        po = 64 * hi
                        nc.tensor.matmul(
                            ps[:, hi * TOK:(hi + 1) * TOK],
                            lhsT=kt_sb[po:po + 64, pair, tb * P:(tb + 1) * P],
                            rhs=qt_sb[po:po + 64, pair, :],
                            start=True, stop=True,
                            tile_position=(po, 0),
                        )
                    nc.scalar.activation(
                        out=et[:, ti, :], in_=ps, func=AF.Exp,
                        bias=maskb_sb[:, tb:tb + 1], scale=1.0)
                denstage = awork.tile([1, 2, TOK], F32, tag="denstage")
                for hi in range(2):
                    h = pair * 2 + hi
                    pc = ctx_psum.tile([HD + 1, TOK], F32, tag="ctx")
                    for tb in range(TCHL):
                        et = et_halves[tb // half]
                        ti = tb % half
                        nc.tensor.matmul(
                            pc,
                            lhsT=vaug_sb[:, tb, h, :],
                            rhs=et[:, ti, hi * TOK:(hi + 1) * TOK],
                            start=(tb == 0), stop=(tb == TCHL - 1),
                        )
                    po = 64 * hi
                    nc.vector.tensor_copy(
                        out=ctxu_sb[po:po + 64, pair, :], in_=pc[0:HD, :])
                    nc.vector.tensor_copy(out=denstage[:, hi, :],
                                          in_=pc[HD:HD + 1, :])
                # per-pair softmax normalization, pipelined behind the next
                # pair's scores: reciprocal runs on 2 partitions, broadcast
                # across 64 partitions via a K=1 outer product on the PE
                den2 = awork.tile([2, TOK], F32, tag="den2")
                nc.sync.dma_start(out=den2, in_=denstage)
                nc.vector.reciprocal(out=den2, in_=den2)
                rcb = awork.tile([2, TOK], BF16, tag="rcb")
                nc.vector.tensor_copy(out=rcb, in_=den2)
                rcflat = awork.tile([1, 2, TOK], BF16, tag="rcflat")
                nc.sync.dma_start(out=rcflat, in_=rcb)
                pb = ctx_psum.tile([P, TOK], F32, tag="ctx")
                for hi in range(2):
                    po = 64 * hi
                    nc.tensor.matmul(pb[po:po + 64, :], lhsT=ones_sb,
                                     rhs=rcflat[:, hi, :],
                                     start=True, stop=True,
                                     tile_position=(0, po))
                rb_sb = awork.tile([P, TOK], F32, tag="rbsb")
                nc.vector.tensor_copy(out=rb_sb, in_=pb)
                nc.vector.tensor_mul(
                    out=ctx_sb[:, pair, :],
                    in0=ctxu_sb[:, pair, :], in1=rb_sb)

        # --- Wo + residual + LN2 ---
        mid = ctx.enter_context(tc.tile_pool(name="mid", bufs=1))
        resid_sb = mid.tile([P, SBLK, D], F32)
        z2t_sb = mid.tile([P, DC, TOK], BF16)
        with tc.tile_pool(name="wo_psum", bufs=2, space="PSUM") as wo_psum, \
             tc.tile_pool(name="wo_wk", bufs=3) as wwork:
            for sb in range(SBLK):
                pw = wo_psum.tile([P, D], F32, tag="wo")
                for dc in range(DC):
                    for oh in range(2):
                        nc.tensor.matmul(
                            pw[:, oh * 512:(oh + 1) * 512],
                            lhsT=ctx_sb[:, dc, sb * P:(sb + 1) * P],
                            rhs=wo_sb[:, dc, oh * 512:(oh + 1) * 512],
                            start=(dc == 0), stop=(dc == DC - 1),
                        )
                rs = resid_sb[:, sb, :]
                nc.vector.tensor_add(out=rs, in0=pw, in1=x_sb[:, sb, :])
                nc.vector.tensor_add(out=rs, in0=rs, in1=bo_sb)
                z2 = wwork.tile([P, D], BF16, tag="z2")
                _ln_tile(nc, wwork, rs, z2, eps_sb)
                for dc in range(DC):
                    pt = wo_psum.tile([P, P], BF16, tag="tp2")
                    nc.tensor.transpose(pt, z2[:, dc * P:(dc + 1) * P], ident)
                    nc.vector.tensor_copy(
                        out=z2t_sb[:, dc, sb * P:(sb + 1) * P], in_=pt)

        # --- FFN (fused: h1 block feeds the second GEMM immediately) ---
        ffn = ctx.enter_context(tc.tile_pool(name="ffn", bufs=1))
        h1t_sb = ffn.tile([P, FB, TOK], BF16)
        with tc.tile_pool(name="ff_psum", bufs=2, space="PSUM") as fa_psum, \
             tc.tile_pool(name="ffb_psum", bufs=4, space="PSUM") as fb_psum, \
             tc.tile_pool(name="ff_w1", bufs=6) as w1pool, \
             tc.tile_pool(name="ff_w2", bufs=6) as w2pool, \
             tc.tile_pool(name="ff_wk", bufs=4) as fwork:
            po_tiles = [fb_psum.tile([P, 512], F32, tag="ffb", name=f"po0_{sb}")
                        for sb in range(SBLK)]
            for fb in range(FB):
                w1t = w1pool.tile([P, DC, P], BF16, tag="w1t")
                nc.gpsimd.dma_start(
                    out=w1t,
                    in_=w1_d[:, fb * P:(fb + 1) * P].rearrange(
                        "(c p) f -> p c f", p=P))
                pf = fa_psum.tile([P, TOK], F32, tag="ffa")
                for dc in range(DC):
                    nc.tensor.matmul(
                        pf, lhsT=w1t[:, dc, :], rhs=z2t_sb[:, dc, :],
                        start=(dc == 0), stop=(dc == DC - 1))
                nc.scalar.activation(
                    out=h1t_sb[:, fb, :], in_=pf, func=AF.Relu,
                    bias=b1_sb[:, fb:fb + 1], scale=1.0)
                w2t = w2pool.tile([P, 512], BF16, tag="w2t")
                nc.gpsimd.dma_start(out=w2t, in_=w2_d[fb * P:(fb + 1) * P, 0:512])
                for sb in range(SBLK):
                    nc.tensor.matmul(
                        po_tiles[sb], lhsT=h1t_sb[:, fb, sb * P:(sb + 1) * P],
                        rhs=w2t, start=(fb == 0), stop=(fb == FB - 1))
            for sb in range(SBLK):
                ot = fwork.tile([P, 512], F32, tag="out")
                nc.vector.tensor_add(out=ot, in0=po_tiles[sb],
                                     in1=resid_sb[:, sb, 0:512])
                nc.vector.tensor_add(out=ot, in0=ot, in1=b2_sb[:, 0:512])
                nc.sync.dma_start(out=out_d[sb * P:(sb + 1) * P, 0:512], in_=ot)
            # second output half: re-stream W2's right half
            po2_tiles = [fb_psum.tile([P, 512], F32, tag="ffb", name=f"po1_{sb}")
                         for sb in range(SBLK)]
            for fb in range(FB):
                w2t = w2pool.tile([P, 512], BF16, tag="w2t")
                nc.gpsimd.dma_start(out=w2t, in_=w2_d[fb * P:(fb + 1) * P, 512:1024])
                for sb in range(SBLK):
                    nc.tensor.matmul(
                        po2_tiles[sb], lhsT=h1t_sb[:, fb, sb * P:(sb + 1) * P],
                        rhs=w2t, start=(fb == 0), stop=(fb == FB - 1))
            for sb in range(SBLK):
                ot = fwork.tile([P, 512], F32, tag="out")
                nc.vector.tensor_add(out=ot, in0=po2_tiles[sb],
                                     in1=resid_sb[:, sb, 512:1024])
                nc.vector.tensor_add(out=ot, in0=ot, in1=b2_sb[:, 512:1024])
                nc.sync.dma_start(out=out_d[sb * P:(sb + 1) * P, 512:1024], in_=ot)

    return nc


_programs = {}
LAST_EXEC_NS = {}


T_PAD = 1152  # compacted key columns (only mask==1 keys kept, padded)


def _get_programs(tkey):
    if "l1" not in _programs:
        l1 = _build_launch1()
        l1.finalize()
        _programs["l1"] = l1
    if ("l2", tkey) not in _programs:
        l2 = _build_launch2(tkey)
        l2.finalize()
        _programs[("l2", tkey)] = l2
    return _programs["l1"], _programs[("l2", tkey)]


def kernel(**inputs):
    inp = {k: np.asarray(v) for k, v in inputs.items()}
    x = inp["x"].astype(np.float32).reshape(B * S, D)
    mask = inp["mask"].astype(np.int32)

    # ---- host-side weight prep (layout + LN-affine folding, fp32 math) ----
    scale = np.float32(1.0 / np.sqrt(HD))
    Wq = inp["Wq"].astype(np.float32).transpose(1, 0, 2).reshape(D, D)
    Wk = inp["Wk"].astype(np.float32).transpose(1, 0, 2).reshape(D, D)
    Wv = inp["Wv"].astype(np.float32).transpose(1, 0, 2).reshape(D, D)
    g1 = inp["ln1_g"].astype(np.float32)
    b1n = inp["ln1_b"].astype(np.float32)
    g2 = inp["ln2_g"].astype(np.float32)
    b2n = inp["ln2_b"].astype(np.float32)

    wq_p = np.ascontiguousarray((g1[:, None] * Wq * scale).astype(bf16_np))
    bq_p = np.ascontiguousarray(
        (b1n @ Wq) * scale + inp["bq"].astype(np.float32).reshape(-1) * scale
    ).astype(np.float32)
    wk_p = np.ascontiguousarray((g1[:, None] * Wk).astype(bf16_np))
    bk_p = ((b1n @ Wk) + inp["bk"].astype(np.float32).reshape(-1)).astype(np.float32)
    wv_p = np.ascontiguousarray((g1[:, None] * Wv).astype(bf16_np))
    bv_p = ((b1n @ Wv) + inp["bv"].astype(np.float32).reshape(-1)).astype(np.float32)
    bv_b = np.ascontiguousarray(np.tile(bv_p[None, :], (P, 1)))

    wo_p = np.ascontiguousarray(inp["Wo"].astype(np.float32).astype(bf16_np))
    bo_b = np.ascontiguousarray(
        np.tile(inp["bo"].astype(np.float32)[None, :], (P, 1)))
    w1_p = np.ascontiguousarray(
        (g2[:, None] * inp["W1"].astype(np.float32)).astype(bf16_np))
    b1_p = ((b2n @ inp["W1"].astype(np.float32))
            + inp["b1"].astype(np.float32)).astype(np.float32)
    w2_p = np.ascontiguousarray(inp["W2"].astype(np.float32).astype(bf16_np))
    b2_b = np.ascontiguousarray(
        np.tile(inp["b2"].astype(np.float32)[None, :], (P, 1)))

    # compaction: keys with mask==0 contribute exactly zero (exp(-1e9) == 0),
    # so only mask==1 columns are kept, padded to T_PAD with mask==0 dummies.
    counts = [int((mask[b] == 1).sum()) for b in range(B)]
    tkey = T_PAD if max(counts) <= T_PAD else S
    l1, l2 = _get_programs(tkey)
    core_ids = list(range(NCORES))
    profile = bool(os.environ.get("KERNEL_PROFILE"))
    kw = {"trace": True} if profile else {}

    # ---- launch 1 ----
    in_maps1 = []
    for c in range(NCORES):
        xc = np.ascontiguousarray(x[c * TOK:(c + 1) * TOK, :])
        in_maps1.append({
            "x": xc, "wq": wq_p, "wk": wk_p, "wv": wv_p,
            "bq": bq_p, "bk": bk_p, "bvb": bv_b,
        })
    r1 = run_bass_kernel_spmd(l1, in_maps1, core_ids, **kw)
    res1 = r1.results

    # ---- host reshuffle: concat K/V shards per batch, compact by mask ----
    kT_b, v_b, mask_b = [], [], []
    for b in range(B):
        kT = np.concatenate([res1[4 * b + i]["kT"] for i in range(4)], axis=1)
        v = np.concatenate([res1[4 * b + i]["v"] for i in range(4)], axis=0)
        if tkey == S:
            kT_b.append(np.ascontiguousarray(kT))
            v_b.append(np.ascontiguousarray(v))
            mask_b.append(np.ascontiguousarray(mask[b]))
        else:
            idx = np.nonzero(mask[b] == 1)[0]
            pad = tkey - len(idx)
            idx_pad = np.concatenate([idx, np.zeros(pad, np.int64)])
            kT_b.append(np.ascontiguousarray(kT[:, idx_pad]))
            v_b.append(np.ascontiguousarray(v[idx_pad, :]))
            mask_b.append(np.concatenate(
                [np.ones(len(idx), np.int32), np.zeros(pad, np.int32)]))

    # ---- launch 2 ----
    in_maps2 = []
    for c in range(NCORES):
        b = c // 4
        in_maps2.append({
            "qT": np.ascontiguousarray(res1[c]["qT"]),
            "kT": kT_b[b],
            "v": v_b[b],
            "mask": mask_b[b],
            "x": np.ascontiguousarray(x[c * TOK:(c + 1) * TOK, :]),
            "wo": wo_p, "bob": bo_b,
            "w1": w1_p, "b1": b1_p, "w2": w2_p, "b2b": b2_b,
        })
    r2 = run_bass_kernel_spmd(l2, in_maps2, core_ids, **kw)
    res2 = r2.results

    if profile:
        LAST_EXEC_NS.clear()
        LAST_EXEC_NS["l1"] = r1.exec_time_ns
        LAST_EXEC_NS["l2"] = r2.exec_time_ns
        LAST_EXEC_NS["l1_trace"] = getattr(r1, "instructions_and_trace", None)
        LAST_EXEC_NS["l2_trace"] = getattr(r2, "instructions_and_trace", None)

    out = np.concatenate([res2[c]["out"] for c in range(NCORES)], axis=0)
    return out.reshape(B, S, D).astype(np.float32)
